# revision 31
# baseline (speedup 1.0000x reference)
"""BoxConv2d Trainium2 kernel (8 NeuronCores, SPMD).

Math: the reference computes, per output channel k = (c, f),
    out[b,k] = interp-row(I) diff, then interp-col diff
where I is the zero-padded integral image of input[b,c].  That whole
pipeline (integral image + fractional box-edge interpolation) is linear
in the input and separable, so it collapses to two dense 128x128
matrix products per image:

    out[b,k] = A_k @ x[b,c] @ B_k^T

with banded "pixel overlap" matrices
    A_k[xo, a] = clamp(xo - a + x_max_k + 1, 0, 1)
                 - clamp(xo - a + x_min_k, 0, 1)
(the overlap length between the box row extent [xo+x_min, xo+x_max+1]
and the pixel row [a, a+1]), and likewise B_k for columns.  A/B are
built on the host from the tiny (C,F) box params; the device does pure
128-contraction matmuls on the PE array.

Sharding: the K = C*F = 128 output channels are split across 8 cores
(16 channels = 4 in_planes per core), so each core reads only its own
4 input planes and input reads are not duplicated chip-wide.

Trace-driven design notes (v7, vs the 52us f32r baseline):
  * bf16 on the wire (x, at, bt, V, out) halves DMA bytes; fp32 PSUM
    accumulation keeps l2 rel err ~3e-3 (gate is 2e-2).
  * Only Scalar+Vector can drain PSUM on TRN2, and that drain (4.2M
    elements) is the binding resource.  PSUM is organized as one pool
    of [128,2048] 4-bank tiles (2 bufs = all 8 banks): 4 matmuls fill
    a tile, one big copy drains it, so per-copy overhead is paid 16x
    instead of 32-64x.
  * The core's DVFS grants ~20.5us of full clock per kernel, the bulk
    of it starting ~21us in; everything before runs at half clock.  A
    couple of dependency-free warmup matmuls right after the preamble
    pull a small early grant, and the schedule keeps the PE/engines
    dense so the big grant covers the whole drain phase.
  * DMA engines fair-share all in-flight transfers, so loads are
    need-ordered c-major on one queue (x is laid out [a,(c,b,j)] so
    pass 1 for channel c needs only that channel's 131KB quads).
  * Stores are 8 contiguous 512KB blocks (two output channels each).
"""

import os
import sys

if "/opt/trn_rl_repo" not in sys.path:
    sys.path.insert(0, "/opt/trn_rl_repo")

import ml_dtypes
import numpy as np

import concourse.bass as bass  # noqa: F401
import concourse.mybir as mybir
import concourse.tile as tile
from concourse import bacc
from concourse.bass_utils import run_bass_kernel_spmd

B, C, F, H, W = 8, 32, 4, 128, 128
NCORES = 8
CPC = C // NCORES  # in_planes per core
KPC = CPC * F      # output channels per core

_DT = mybir.dt.bfloat16
_NP_DT = ml_dtypes.bfloat16

_NC_CACHE = {}
LAST_RESULT = None


def _build_nc():
    nc = bacc.Bacc(
        "TRN2", target_bir_lowering=False, debug=False, num_devices=NCORES
    )
    # x[a, (c, b, j)]: c-major so pass 1 of channel c loads only its own
    # two 131KB b-quads; rows are 1KB contiguous in DRAM
    x_p = nc.declare_dram_parameter("x", [H, CPC * B * W], _DT, isOutput=False)
    at_p = nc.declare_dram_parameter(
        "at", [H, CPC * F * H], _DT, isOutput=False)
    bt_p = nc.declare_dram_parameter(
        "bt", [W, CPC * F * W], _DT, isOutput=False)
    # outT[kl, yo, (b, xo)]: stores are contiguous 512KB kl-pair blocks
    out_p = nc.declare_dram_parameter(
        "outT", [KPC, W, B * H], _DT, isOutput=True)

    with tile.TileContext(nc) as tc:
        with (
            tc.tile_pool(name="const", bufs=1) as cpool,
            tc.tile_pool(name="vall", bufs=3) as vpool,
            tc.tile_pool(name="osb", bufs=3) as opool,
            tc.tile_pool(name="pv", bufs=2, space="PSUM") as pvpool,
            tc.tile_pool(name="po", bufs=2, space="PSUM") as popool,
        ):
            # Few large need-ordered loads: per-DMA queue cost is ~0.7us
            # regardless of size (128 descriptors either way), and the
            # DMA engines fair-share all in-flight transfers, so pairing
            # channels halves both issue time and completion spread.
            at_t2 = [cpool.tile([128, 2 * F * H], _DT, name=f"at{i}",
                                tag=f"at{i}") for i in range(2)]
            bt_t2 = [cpool.tile([128, 2 * F * W], _DT, name=f"bt{i}",
                                tag=f"bt{i}") for i in range(2)]
            # x per channel (deps are tile-granular, so per-c tiles let
            # the first pass-1 matmul wait only on at[c01] + x[c0])
            x_t = [cpool.tile([128, B * W], _DT, name=f"x{c}",
                              tag=f"x{c}") for c in range(CPC)]

            def at_c(c):
                return at_t2[c // 2][:, (c % 2) * F * H:(c % 2 + 1) * F * H]

            def bt_cf(c, f):
                o = ((c % 2) * F + f) * W
                return bt_t2[c // 2][:, o:o + W]

            def x_bc(b, c):
                return x_t[c][:, b * W:(b + 1) * W]

            at_r = at_p[:].rearrange("a (i cfx) -> a i cfx", i=2)
            bt_r = bt_p[:].rearrange("j (i cfy) -> j i cfy", i=2)
            x_r = x_p[:].rearrange("a (c bj) -> a c bj", c=CPC)

            nc.sync.dma_start(at_t2[0][:], at_r[:, 0])
            nc.sync.dma_start(x_t[0][:], x_r[:, 0])
            nc.sync.dma_start(x_t[1][:], x_r[:, 1])
            nc.sync.dma_start(bt_t2[0][:], bt_r[:, 0])
            nc.sync.dma_start(at_t2[1][:], at_r[:, 1])
            nc.sync.dma_start(x_t[2][:], x_r[:, 2])
            nc.sync.dma_start(x_t[3][:], x_r[:, 3])
            nc.sync.dma_start(bt_t2[1][:], bt_r[:, 1])

            # dependency-free warmup matmuls: PE activity starts the
            # DVFS clock-up while the first loads are still in flight
            # dependency-free warmup: a hard burst of activity on PE,
            # GpSimd and both copy engines right after the preamble, to
            # pull the DVFS full-clock grant (fixed ~20.5us budget)
            # earlier -- pre-grant everything runs at half clock
            warm = cpool.tile([128, 512], _DT, name="warm", tag="warm")
            warm2 = cpool.tile([128, 512], _DT, name="warm2", tag="warm2")
            nc.gpsimd.memset(warm[:], 0.0)
            for w in range(3):
                nc.scalar.copy(warm2[:], warm[:])
                nc.vector.tensor_copy(warm2[:], warm[:])
                nc.gpsimd.memset(warm2[:], 0.0)
            for w in range(6):
                w_ps = pvpool.tile([128, 2 * F * H], mybir.dt.float32,
                                   name=f"wps{w}", tag="vps")
                nc.tensor.matmul(
                    w_ps[:, :512], lhsT=warm[:, :128], rhs=warm[:],
                    start=True, stop=True)

            # PSUM->SBUF copies: only Scalar and Vector can read PSUM;
            # alternate with a slight lean to the faster Scalar (17:15)
            cp_i = [0]

            def copy(dst, src):
                i = cp_i[0]
                cp_i[0] += 1
                if i % 2 == 1 and i < 30:
                    nc.vector.tensor_copy(dst, src)
                else:
                    nc.scalar.copy(dst, src)

            v_all = [None] * CPC

            def emit_pass1(c, bp):
                # two 512-col matmuls into one 2-bank PSUM tile, then one
                # 1024-col copy into V[c][j, (f, b, xo)]
                if bp == 0:
                    v_all[c] = vpool.tile(
                        [128, F * B * H], _DT, name=f"v{c}", tag="vall")
                v_ps = pvpool.tile([128, 2 * F * H], mybir.dt.float32,
                                   name=f"vps{c}{bp}", tag="vps")
                for i in range(2):
                    b = 2 * bp + i
                    nc.tensor.matmul(
                        v_ps[:, i * F * H:(i + 1) * F * H],
                        lhsT=x_bc(b, c),
                        rhs=at_c(c),
                        start=True,
                        stop=True,
                    )
                src = v_ps[:].rearrange("p (i f xo) -> p i f xo", i=2, f=F)
                dst = v_all[c][:].rearrange(
                    "p (f b xo) -> p f b xo", f=F, b=B)
                copy(dst[:, :, 2 * bp:2 * bp + 2, :],
                     src.rearrange("p i f xo -> p f i xo"))

            o_pair = [None]

            def emit_pass2(c, f):
                kl = c * F + f
                # O[yo, (b, xo)] for all 8 batches: 2x N=512 matmuls
                # into one 2-bank tile, one copy; two channels share an
                # o_sb tile so each store is one contiguous 512KB block
                o_ps = popool.tile([128, B * H], mybir.dt.float32,
                                   name=f"ops{kl}", tag="ops")
                for i in range(2):
                    nc.tensor.matmul(
                        o_ps[:, i * 512:(i + 1) * 512],
                        lhsT=bt_cf(c, f),
                        rhs=v_all[c][:, f * B * H + i * 512:
                                     f * B * H + (i + 1) * 512],
                        start=True,
                        stop=True,
                    )
                if f % 2 == 0:
                    o_pair[0] = opool.tile([128, 2 * B * H], _DT,
                                           name=f"osb{kl}", tag="osb")
                o_sb = o_pair[0]
                copy(o_sb[:, (f % 2) * B * H:(f % 2 + 1) * B * H], o_ps[:])
                if f % 2 == 1:
                    # dst iterates [kl, yo, bxo]; flip to [yo, kl, bxo]
                    # to match the SBUF tile's element order
                    dst = out_p[kl - 1:kl + 1].rearrange(
                        "f yo bx -> yo f bx")
                    nc.sync.dma_start(dst, o_sb[:])

            # software pipeline: pass 2 of channel c-1 interleaves with
            # pass 1 of channel c at matching granularity, keeping the PE
            # dense and the drain engines and store stream busy from ~10us
            for bp in range(B // 2):
                emit_pass1(0, bp)
            for c in range(1, CPC):
                for k in range(4):
                    emit_pass2(c - 1, k)
                    emit_pass1(c, k)
            for f in range(F):
                emit_pass2(CPC - 1, f)
    nc.finalize()
    return nc


def _get_nc():
    if "nc" not in _NC_CACHE:
        _NC_CACHE["nc"] = _build_nc()
    return _NC_CACHE["nc"]


def _overlap_mats(lo, hi):
    """(K, out, in) pixel-overlap matrices for a 128-wide axis."""
    t = np.arange(128, dtype=np.float64)
    d = t[:, None] - t[None, :]  # out - in
    lo = lo.astype(np.float64)[:, None, None]
    hi = hi.astype(np.float64)[:, None, None]
    m = np.clip(d[None] + hi + 1.0, 0.0, 1.0) - np.clip(d[None] + lo, 0.0, 1.0)
    return m.astype(np.float32)


def _make_in_maps(input, x_min, x_max, y_min, y_max):
    A = _overlap_mats(x_min.reshape(-1), x_max.reshape(-1))   # (K, xo, a)
    Bm = _overlap_mats(y_min.reshape(-1), y_max.reshape(-1))  # (K, yo, j)
    in_maps = []
    for m in range(NCORES):
        cs = slice(CPC * m, CPC * (m + 1))
        ks = slice(KPC * m, KPC * (m + 1))
        # x[a, (c, b, j)]
        xm = input[:, cs].transpose(2, 1, 0, 3).reshape(H, CPC * B * W)
        # at[a, (c, f, xo)] = A[k=c*F+f, xo, a]
        at = A[ks].reshape(CPC, F, H, H).transpose(3, 0, 1, 2)
        bt = Bm[ks].reshape(CPC, F, W, W).transpose(3, 0, 1, 2)
        in_maps.append({
            "x": np.ascontiguousarray(xm).astype(_NP_DT),
            "at": np.ascontiguousarray(
                at.reshape(H, CPC * F * H)).astype(_NP_DT),
            "bt": np.ascontiguousarray(
                bt.reshape(W, CPC * F * W)).astype(_NP_DT),
        })
    return in_maps


def _assemble(results):
    out = np.empty((B, C * F, H, W), np.float32)
    for m in range(NCORES):
        # outT[kl, yo, b, xo] -> out[b, kl, xo, yo]
        o = results[m]["outT"].reshape(KPC, W, B, H).astype(np.float32)
        out[:, KPC * m:KPC * (m + 1)] = o.transpose(2, 0, 3, 1)
    return out


def _run(inputs, trace=False):
    global LAST_RESULT
    nc = _get_nc()
    in_maps = _make_in_maps(**inputs)
    LAST_RESULT = run_bass_kernel_spmd(
        nc, in_maps, list(range(NCORES)), trace=trace
    )
    return _assemble(LAST_RESULT.results)


def kernel(input, x_min, x_max, y_min, y_max):
    return _run({
        "input": np.asarray(input, dtype=np.float32),
        "x_min": np.asarray(x_min, dtype=np.float32),
        "x_max": np.asarray(x_max, dtype=np.float32),
        "y_min": np.asarray(y_min, dtype=np.float32),
        "y_max": np.asarray(y_max, dtype=np.float32),
    })


# revision 32
# speedup vs baseline: 1.0035x; 1.0035x over previous
"""BoxConv2d Trainium2 kernel (8 NeuronCores, SPMD).

Math: the reference computes, per output channel k = (c, f),
    out[b,k] = interp-row(I) diff, then interp-col diff
where I is the zero-padded integral image of input[b,c].  That whole
pipeline (integral image + fractional box-edge interpolation) is linear
in the input and separable, so it collapses to two dense 128x128
matrix products per image:

    out[b,k] = A_k @ x[b,c] @ B_k^T

with banded "pixel overlap" matrices
    A_k[xo, a] = clamp(xo - a + x_max_k + 1, 0, 1)
                 - clamp(xo - a + x_min_k, 0, 1)
(the overlap length between the box row extent [xo+x_min, xo+x_max+1]
and the pixel row [a, a+1]), and likewise B_k for columns.  A/B are
built on the host from the tiny (C,F) box params; the device does pure
128-contraction matmuls on the PE array.

Sharding: the K = C*F = 128 output channels are split across 8 cores
(16 channels = 4 in_planes per core), so each core reads only its own
4 input planes and input reads are not duplicated chip-wide.

Trace-driven design notes (v7, vs the 52us f32r baseline):
  * bf16 on the wire (x, at, bt, V, out) halves DMA bytes; fp32 PSUM
    accumulation keeps l2 rel err ~3e-3 (gate is 2e-2).
  * Only Scalar+Vector can drain PSUM on TRN2, and that drain (4.2M
    elements) is the binding resource.  PSUM is organized as one pool
    of [128,2048] 4-bank tiles (2 bufs = all 8 banks): 4 matmuls fill
    a tile, one big copy drains it, so per-copy overhead is paid 16x
    instead of 32-64x.
  * The core's DVFS grants ~20.5us of full clock per kernel, the bulk
    of it starting ~21us in; everything before runs at half clock.  A
    couple of dependency-free warmup matmuls right after the preamble
    pull a small early grant, and the schedule keeps the PE/engines
    dense so the big grant covers the whole drain phase.
  * DMA engines fair-share all in-flight transfers, so loads are
    need-ordered c-major on one queue (x is laid out [a,(c,b,j)] so
    pass 1 for channel c needs only that channel's 131KB quads).
  * Stores are 8 contiguous 512KB blocks (two output channels each).
"""

import os
import sys

if "/opt/trn_rl_repo" not in sys.path:
    sys.path.insert(0, "/opt/trn_rl_repo")

import ml_dtypes
import numpy as np

import concourse.bass as bass  # noqa: F401
import concourse.mybir as mybir
import concourse.tile as tile
from concourse import bacc
from concourse.bass_utils import run_bass_kernel_spmd

B, C, F, H, W = 8, 32, 4, 128, 128
NCORES = 8
CPC = C // NCORES  # in_planes per core
KPC = CPC * F      # output channels per core

_DT = mybir.dt.bfloat16
_NP_DT = ml_dtypes.bfloat16

_NC_CACHE = {}
LAST_RESULT = None


def _build_nc():
    nc = bacc.Bacc(
        "TRN2", target_bir_lowering=False, debug=False, num_devices=NCORES
    )
    # x[a, (c, b, j)]: c-major so pass 1 of channel c loads only its own
    # two 131KB b-quads; rows are 1KB contiguous in DRAM
    x_p = nc.declare_dram_parameter("x", [H, CPC * B * W], _DT, isOutput=False)
    at_p = nc.declare_dram_parameter(
        "at", [H, CPC * F * H], _DT, isOutput=False)
    bt_p = nc.declare_dram_parameter(
        "bt", [W, CPC * F * W], _DT, isOutput=False)
    # outT[kl, yo, (b, xo)]: stores are contiguous 512KB kl-pair blocks
    out_p = nc.declare_dram_parameter(
        "outT", [KPC, W, B * H], _DT, isOutput=True)

    with tile.TileContext(nc) as tc:
        with (
            tc.tile_pool(name="const", bufs=1) as cpool,
            tc.tile_pool(name="vall", bufs=3) as vpool,
            tc.tile_pool(name="osb", bufs=3) as opool,
            tc.tile_pool(name="pv", bufs=2, space="PSUM") as pvpool,
            tc.tile_pool(name="po", bufs=2, space="PSUM") as popool,
        ):
            # Few large need-ordered loads: per-DMA queue cost is ~0.7us
            # regardless of size (128 descriptors either way), and the
            # DMA engines fair-share all in-flight transfers, so pairing
            # channels halves both issue time and completion spread.
            at_t2 = [cpool.tile([128, 2 * F * H], _DT, name=f"at{i}",
                                tag=f"at{i}") for i in range(2)]
            bt_t2 = [cpool.tile([128, 2 * F * W], _DT, name=f"bt{i}",
                                tag=f"bt{i}") for i in range(2)]
            # x per channel (deps are tile-granular, so per-c tiles let
            # the first pass-1 matmul wait only on at[c01] + x[c0])
            x_t = [cpool.tile([128, B * W], _DT, name=f"x{c}",
                              tag=f"x{c}") for c in range(CPC)]

            def at_c(c):
                return at_t2[c // 2][:, (c % 2) * F * H:(c % 2 + 1) * F * H]

            def bt_cf(c, f):
                o = ((c % 2) * F + f) * W
                return bt_t2[c // 2][:, o:o + W]

            def x_bc(b, c):
                return x_t[c][:, b * W:(b + 1) * W]

            at_r = at_p[:].rearrange("a (i cfx) -> a i cfx", i=2)
            bt_r = bt_p[:].rearrange("j (i cfy) -> j i cfy", i=2)
            x_r = x_p[:].rearrange("a (c bj) -> a c bj", c=CPC)

            nc.sync.dma_start(at_t2[0][:], at_r[:, 0])
            nc.sync.dma_start(x_t[0][:], x_r[:, 0])
            nc.sync.dma_start(x_t[1][:], x_r[:, 1])
            nc.sync.dma_start(bt_t2[0][:], bt_r[:, 0])
            nc.sync.dma_start(at_t2[1][:], at_r[:, 1])
            nc.sync.dma_start(x_t[2][:], x_r[:, 2])
            nc.sync.dma_start(x_t[3][:], x_r[:, 3])
            nc.sync.dma_start(bt_t2[1][:], bt_r[:, 1])

            # dependency-free warmup matmuls: PE activity starts the
            # DVFS clock-up while the first loads are still in flight
            # dependency-free warmup: a hard burst of activity on PE,
            # GpSimd and both copy engines right after the preamble, to
            # pull the DVFS full-clock grant (fixed ~20.5us budget)
            # earlier -- pre-grant everything runs at half clock
            warm = cpool.tile([128, 512], _DT, name="warm", tag="warm")
            warm2 = cpool.tile([128, 512], _DT, name="warm2", tag="warm2")
            nc.gpsimd.memset(warm[:], 0.0)
            for w in range(5):
                nc.scalar.copy(warm2[:], warm[:])
                nc.vector.tensor_copy(warm2[:], warm[:])
                nc.gpsimd.memset(warm2[:], 0.0)
            for w in range(6):
                w_ps = pvpool.tile([128, 2 * F * H], mybir.dt.float32,
                                   name=f"wps{w}", tag="vps")
                nc.tensor.matmul(
                    w_ps[:, :512], lhsT=warm[:, :128], rhs=warm[:],
                    start=True, stop=True)

            # PSUM->SBUF copies: only Scalar and Vector can read PSUM;
            # alternate with a slight lean to the faster Scalar (17:15)
            cp_i = [0]

            def copy(dst, src):
                i = cp_i[0]
                cp_i[0] += 1
                if i % 2 == 1 and i < 30:
                    nc.vector.tensor_copy(dst, src)
                else:
                    nc.scalar.copy(dst, src)

            v_all = [None] * CPC

            def emit_pass1(c, bp):
                # two 512-col matmuls into one 2-bank PSUM tile, then one
                # 1024-col copy into V[c][j, (f, b, xo)]
                if bp == 0:
                    v_all[c] = vpool.tile(
                        [128, F * B * H], _DT, name=f"v{c}", tag="vall")
                v_ps = pvpool.tile([128, 2 * F * H], mybir.dt.float32,
                                   name=f"vps{c}{bp}", tag="vps")
                for i in range(2):
                    b = 2 * bp + i
                    nc.tensor.matmul(
                        v_ps[:, i * F * H:(i + 1) * F * H],
                        lhsT=x_bc(b, c),
                        rhs=at_c(c),
                        start=True,
                        stop=True,
                    )
                src = v_ps[:].rearrange("p (i f xo) -> p i f xo", i=2, f=F)
                dst = v_all[c][:].rearrange(
                    "p (f b xo) -> p f b xo", f=F, b=B)
                copy(dst[:, :, 2 * bp:2 * bp + 2, :],
                     src.rearrange("p i f xo -> p f i xo"))

            o_pair = [None]

            def emit_pass2(c, f):
                kl = c * F + f
                # O[yo, (b, xo)] for all 8 batches: 2x N=512 matmuls
                # into one 2-bank tile, one copy; two channels share an
                # o_sb tile so each store is one contiguous 512KB block
                o_ps = popool.tile([128, B * H], mybir.dt.float32,
                                   name=f"ops{kl}", tag="ops")
                for i in range(2):
                    nc.tensor.matmul(
                        o_ps[:, i * 512:(i + 1) * 512],
                        lhsT=bt_cf(c, f),
                        rhs=v_all[c][:, f * B * H + i * 512:
                                     f * B * H + (i + 1) * 512],
                        start=True,
                        stop=True,
                    )
                if f % 2 == 0:
                    o_pair[0] = opool.tile([128, 2 * B * H], _DT,
                                           name=f"osb{kl}", tag="osb")
                o_sb = o_pair[0]
                copy(o_sb[:, (f % 2) * B * H:(f % 2 + 1) * B * H], o_ps[:])
                if f % 2 == 1:
                    # dst iterates [kl, yo, bxo]; flip to [yo, kl, bxo]
                    # to match the SBUF tile's element order
                    dst = out_p[kl - 1:kl + 1].rearrange(
                        "f yo bx -> yo f bx")
                    nc.sync.dma_start(dst, o_sb[:])

            # software pipeline: pass 2 of channel c-1 interleaves with
            # pass 1 of channel c at matching granularity, keeping the PE
            # dense and the drain engines and store stream busy from ~10us
            for bp in range(B // 2):
                emit_pass1(0, bp)
            for c in range(1, CPC):
                for k in range(4):
                    emit_pass2(c - 1, k)
                    emit_pass1(c, k)
            for f in range(F):
                emit_pass2(CPC - 1, f)
    nc.finalize()
    return nc


def _get_nc():
    if "nc" not in _NC_CACHE:
        _NC_CACHE["nc"] = _build_nc()
    return _NC_CACHE["nc"]


def _overlap_mats(lo, hi):
    """(K, out, in) pixel-overlap matrices for a 128-wide axis."""
    t = np.arange(128, dtype=np.float64)
    d = t[:, None] - t[None, :]  # out - in
    lo = lo.astype(np.float64)[:, None, None]
    hi = hi.astype(np.float64)[:, None, None]
    m = np.clip(d[None] + hi + 1.0, 0.0, 1.0) - np.clip(d[None] + lo, 0.0, 1.0)
    return m.astype(np.float32)


def _make_in_maps(input, x_min, x_max, y_min, y_max):
    A = _overlap_mats(x_min.reshape(-1), x_max.reshape(-1))   # (K, xo, a)
    Bm = _overlap_mats(y_min.reshape(-1), y_max.reshape(-1))  # (K, yo, j)
    in_maps = []
    for m in range(NCORES):
        cs = slice(CPC * m, CPC * (m + 1))
        ks = slice(KPC * m, KPC * (m + 1))
        # x[a, (c, b, j)]
        xm = input[:, cs].transpose(2, 1, 0, 3).reshape(H, CPC * B * W)
        # at[a, (c, f, xo)] = A[k=c*F+f, xo, a]
        at = A[ks].reshape(CPC, F, H, H).transpose(3, 0, 1, 2)
        bt = Bm[ks].reshape(CPC, F, W, W).transpose(3, 0, 1, 2)
        in_maps.append({
            "x": np.ascontiguousarray(xm).astype(_NP_DT),
            "at": np.ascontiguousarray(
                at.reshape(H, CPC * F * H)).astype(_NP_DT),
            "bt": np.ascontiguousarray(
                bt.reshape(W, CPC * F * W)).astype(_NP_DT),
        })
    return in_maps


def _assemble(results):
    out = np.empty((B, C * F, H, W), np.float32)
    for m in range(NCORES):
        # outT[kl, yo, b, xo] -> out[b, kl, xo, yo]
        o = results[m]["outT"].reshape(KPC, W, B, H).astype(np.float32)
        out[:, KPC * m:KPC * (m + 1)] = o.transpose(2, 0, 3, 1)
    return out


def _run(inputs, trace=False):
    global LAST_RESULT
    nc = _get_nc()
    in_maps = _make_in_maps(**inputs)
    LAST_RESULT = run_bass_kernel_spmd(
        nc, in_maps, list(range(NCORES)), trace=trace
    )
    return _assemble(LAST_RESULT.results)


def kernel(input, x_min, x_max, y_min, y_max):
    return _run({
        "input": np.asarray(input, dtype=np.float32),
        "x_min": np.asarray(x_min, dtype=np.float32),
        "x_max": np.asarray(x_max, dtype=np.float32),
        "y_min": np.asarray(y_min, dtype=np.float32),
        "y_max": np.asarray(y_max, dtype=np.float32),
    })


# revision 36
# speedup vs baseline: 1.0312x; 1.0277x over previous
"""BoxConv2d Trainium2 kernel (8 NeuronCores, SPMD).

Math: the reference computes, per output channel k = (c, f),
    out[b,k] = interp-row(I) diff, then interp-col diff
where I is the zero-padded integral image of input[b,c].  That whole
pipeline (integral image + fractional box-edge interpolation) is linear
in the input and separable, so it collapses to two dense 128x128
matrix products per image:

    out[b,k] = A_k @ x[b,c] @ B_k^T

with banded "pixel overlap" matrices
    A_k[xo, a] = clamp(xo - a + x_max_k + 1, 0, 1)
                 - clamp(xo - a + x_min_k, 0, 1)
(the overlap length between the box row extent [xo+x_min, xo+x_max+1]
and the pixel row [a, a+1]), and likewise B_k for columns.  A/B are
built on the host from the tiny (C,F) box params; the device does pure
128-contraction matmuls on the PE array.

Sharding: the K = C*F = 128 output channels are split across 8 cores
(16 channels = 4 in_planes per core), so each core reads only its own
4 input planes and input reads are not duplicated chip-wide.

Trace-driven design notes (vs the 52us f32r baseline):
  * bf16 on the wire (x, at, bt, V, out) halves DMA bytes; fp32 PSUM
    accumulation keeps l2 rel err ~3e-3 (gate is 2e-2).
  * Only Scalar+Vector can drain PSUM on TRN2, and that drain (4.2M
    elements, ~18.6us of engine time each) is the binding resource.
    PSUM runs as two double-buffered [128,1024] 2-bank pools (pass 1 /
    pass 2), each tile filled by two 512-col matmuls and drained by
    one 1024-col cast-copy, alternated across both engines.
  * The core's DVFS grants ~20.5us of full clock per NEFF, normally
    starting ~21us in; everything before runs at half clock.  A
    dependency-free warmup burst on all four engines right after the
    preamble pulls the grant up to ~15.5-16us, which is worth 2-4us
    end to end.  (A fixed ~6us preamble and a ~8us semaphore-reset
    epilogue are framework overhead visible in the measured time.)
  * DMA engines fair-share all in-flight transfers (completion order
    is nearly issue-order-independent), so loads are few, large, and
    need-ordered on one queue: x is laid out [a,(c,b,j)] c-major so
    pass 1 of channel c waits only on its own 262KB tile.
  * Stores are 16 contiguous 256KB blocks riding the same queue,
    paced by the drain, finishing inside the full-clock window.
"""

import os
import sys

if "/opt/trn_rl_repo" not in sys.path:
    sys.path.insert(0, "/opt/trn_rl_repo")

import ml_dtypes
import numpy as np

import concourse.bass as bass  # noqa: F401
import concourse.mybir as mybir
import concourse.tile as tile
from concourse import bacc
from concourse.bass_utils import run_bass_kernel_spmd

B, C, F, H, W = 8, 32, 4, 128, 128
NCORES = 8
CPC = C // NCORES  # in_planes per core
KPC = CPC * F      # output channels per core

_DT = mybir.dt.bfloat16
_NP_DT = ml_dtypes.bfloat16

_NC_CACHE = {}
LAST_RESULT = None


def _build_nc():
    nc = bacc.Bacc(
        "TRN2", target_bir_lowering=False, debug=False, num_devices=NCORES
    )
    # x[a, (c, b, j)]: c-major so pass 1 of channel c loads only its own
    # two 131KB b-quads; rows are 1KB contiguous in DRAM
    x_p = nc.declare_dram_parameter("x", [H, CPC * B * W], _DT, isOutput=False)
    at_p = nc.declare_dram_parameter(
        "at", [H, CPC * F * H], _DT, isOutput=False)
    bt_p = nc.declare_dram_parameter(
        "bt", [W, CPC * F * W], _DT, isOutput=False)
    # outT[kl, yo, (b, xo)]: stores are contiguous 512KB kl-pair blocks
    out_p = nc.declare_dram_parameter(
        "outT", [KPC, W, B * H], _DT, isOutput=True)

    with tile.TileContext(nc) as tc:
        with (
            tc.tile_pool(name="const", bufs=1) as cpool,
            tc.tile_pool(name="vall", bufs=3) as vpool,
            tc.tile_pool(name="osb", bufs=6) as opool,
            tc.tile_pool(name="pv", bufs=2, space="PSUM") as pvpool,
            tc.tile_pool(name="po", bufs=2, space="PSUM") as popool,
        ):
            # Few large need-ordered loads: per-DMA queue cost is ~0.7us
            # regardless of size (128 descriptors either way), and the
            # DMA engines fair-share all in-flight transfers, so pairing
            # channels halves both issue time and completion spread.
            at_t2 = [cpool.tile([128, 2 * F * H], _DT, name=f"at{i}",
                                tag=f"at{i}") for i in range(2)]
            bt_t2 = [cpool.tile([128, 2 * F * W], _DT, name=f"bt{i}",
                                tag=f"bt{i}") for i in range(2)]
            # x per channel (deps are tile-granular, so per-c tiles let
            # the first pass-1 matmul wait only on at[c01] + x[c0])
            x_t = [cpool.tile([128, B * W], _DT, name=f"x{c}",
                              tag=f"x{c}") for c in range(CPC)]

            def at_c(c):
                return at_t2[c // 2][:, (c % 2) * F * H:(c % 2 + 1) * F * H]

            def bt_cf(c, f):
                o = ((c % 2) * F + f) * W
                return bt_t2[c // 2][:, o:o + W]

            def x_bc(b, c):
                return x_t[c][:, b * W:(b + 1) * W]

            at_r = at_p[:].rearrange("a (i cfx) -> a i cfx", i=2)
            bt_r = bt_p[:].rearrange("j (i cfy) -> j i cfy", i=2)
            x_r = x_p[:].rearrange("a (c bj) -> a c bj", c=CPC)

            nc.sync.dma_start(at_t2[0][:], at_r[:, 0])
            nc.sync.dma_start(x_t[0][:], x_r[:, 0])
            nc.sync.dma_start(x_t[1][:], x_r[:, 1])
            nc.sync.dma_start(bt_t2[0][:], bt_r[:, 0])
            nc.sync.dma_start(at_t2[1][:], at_r[:, 1])
            nc.sync.dma_start(x_t[2][:], x_r[:, 2])
            nc.sync.dma_start(x_t[3][:], x_r[:, 3])
            nc.sync.dma_start(bt_t2[1][:], bt_r[:, 1])

            # dependency-free warmup matmuls: PE activity starts the
            # DVFS clock-up while the first loads are still in flight
            # dependency-free warmup: a hard burst of activity on PE,
            # GpSimd and both copy engines right after the preamble, to
            # pull the DVFS full-clock grant (fixed ~20.5us budget)
            # earlier -- pre-grant everything runs at half clock
            warm = cpool.tile([128, 512], _DT, name="warm", tag="warm")
            warm2 = cpool.tile([128, 512], _DT, name="warm2", tag="warm2")
            nc.gpsimd.memset(warm[:], 0.0)
            for w in range(3):
                nc.scalar.copy(warm2[:], warm[:])
                nc.vector.tensor_copy(warm2[:], warm[:])
                nc.gpsimd.memset(warm2[:], 0.0)
            for w in range(6):
                w_ps = pvpool.tile([128, 2 * F * H], mybir.dt.float32,
                                   name=f"wps{w}", tag="vps")
                nc.tensor.matmul(
                    w_ps[:, :512], lhsT=warm[:, :128], rhs=warm[:],
                    start=True, stop=True)

            # PSUM->SBUF copies: only Scalar and Vector can read PSUM;
            # alternate with a slight lean to the faster Scalar (17:15)
            cp_i = [0]

            def copy(dst, src):
                i = cp_i[0]
                cp_i[0] += 1
                if i % 2 == 1 and i < 30:
                    nc.vector.tensor_copy(dst, src)
                else:
                    nc.scalar.copy(dst, src)

            v_all = [None] * CPC

            def emit_pass1(c, bp):
                # two 512-col matmuls into one 2-bank PSUM tile, then one
                # 1024-col copy into V[c][j, (f, b, xo)]
                if bp == 0:
                    v_all[c] = vpool.tile(
                        [128, F * B * H], _DT, name=f"v{c}", tag="vall")
                v_ps = pvpool.tile([128, 2 * F * H], mybir.dt.float32,
                                   name=f"vps{c}{bp}", tag="vps")
                for i in range(2):
                    b = 2 * bp + i
                    nc.tensor.matmul(
                        v_ps[:, i * F * H:(i + 1) * F * H],
                        lhsT=x_bc(b, c),
                        rhs=at_c(c),
                        start=True,
                        stop=True,
                    )
                src = v_ps[:].rearrange("p (i f xo) -> p i f xo", i=2, f=F)
                dst = v_all[c][:].rearrange(
                    "p (f b xo) -> p f b xo", f=F, b=B)
                copy(dst[:, :, 2 * bp:2 * bp + 2, :],
                     src.rearrange("p i f xo -> p f i xo"))

            def emit_pass2(c, f):
                kl = c * F + f
                # O[yo, (b, xo)] for all 8 batches: 2x N=512 matmuls
                # into one 2-bank tile, one copy, one 256KB store
                o_ps = popool.tile([128, B * H], mybir.dt.float32,
                                   name=f"ops{kl}", tag="ops")
                for i in range(2):
                    nc.tensor.matmul(
                        o_ps[:, i * 512:(i + 1) * 512],
                        lhsT=bt_cf(c, f),
                        rhs=v_all[c][:, f * B * H + i * 512:
                                     f * B * H + (i + 1) * 512],
                        start=True,
                        stop=True,
                    )
                o_sb = opool.tile([128, B * H], _DT,
                                  name=f"osb{kl}", tag="osb")
                copy(o_sb[:], o_ps[:])
                nc.sync.dma_start(out_p[kl], o_sb[:])

            # software pipeline: pass 2 of channel c-1 interleaves with
            # pass 1 of channel c at matching granularity, keeping the PE
            # dense and the drain engines and store stream busy from ~10us
            for bp in range(B // 2):
                emit_pass1(0, bp)
            for c in range(1, CPC):
                for k in range(4):
                    emit_pass2(c - 1, k)
                    emit_pass1(c, k)
            for f in range(F):
                emit_pass2(CPC - 1, f)
    nc.finalize()
    return nc


def _get_nc():
    if "nc" not in _NC_CACHE:
        _NC_CACHE["nc"] = _build_nc()
    return _NC_CACHE["nc"]


def _overlap_mats(lo, hi):
    """(K, out, in) pixel-overlap matrices for a 128-wide axis."""
    t = np.arange(128, dtype=np.float64)
    d = t[:, None] - t[None, :]  # out - in
    lo = lo.astype(np.float64)[:, None, None]
    hi = hi.astype(np.float64)[:, None, None]
    m = np.clip(d[None] + hi + 1.0, 0.0, 1.0) - np.clip(d[None] + lo, 0.0, 1.0)
    return m.astype(np.float32)


def _make_in_maps(input, x_min, x_max, y_min, y_max):
    A = _overlap_mats(x_min.reshape(-1), x_max.reshape(-1))   # (K, xo, a)
    Bm = _overlap_mats(y_min.reshape(-1), y_max.reshape(-1))  # (K, yo, j)
    in_maps = []
    for m in range(NCORES):
        cs = slice(CPC * m, CPC * (m + 1))
        ks = slice(KPC * m, KPC * (m + 1))
        # x[a, (c, b, j)]
        xm = input[:, cs].transpose(2, 1, 0, 3).reshape(H, CPC * B * W)
        # at[a, (c, f, xo)] = A[k=c*F+f, xo, a]
        at = A[ks].reshape(CPC, F, H, H).transpose(3, 0, 1, 2)
        bt = Bm[ks].reshape(CPC, F, W, W).transpose(3, 0, 1, 2)
        in_maps.append({
            "x": np.ascontiguousarray(xm).astype(_NP_DT),
            "at": np.ascontiguousarray(
                at.reshape(H, CPC * F * H)).astype(_NP_DT),
            "bt": np.ascontiguousarray(
                bt.reshape(W, CPC * F * W)).astype(_NP_DT),
        })
    return in_maps


def _assemble(results):
    out = np.empty((B, C * F, H, W), np.float32)
    for m in range(NCORES):
        # outT[kl, yo, b, xo] -> out[b, kl, xo, yo]
        o = results[m]["outT"].reshape(KPC, W, B, H).astype(np.float32)
        out[:, KPC * m:KPC * (m + 1)] = o.transpose(2, 0, 3, 1)
    return out


def _run(inputs, trace=False):
    global LAST_RESULT
    nc = _get_nc()
    in_maps = _make_in_maps(**inputs)
    LAST_RESULT = run_bass_kernel_spmd(
        nc, in_maps, list(range(NCORES)), trace=trace
    )
    return _assemble(LAST_RESULT.results)


def kernel(input, x_min, x_max, y_min, y_max):
    return _run({
        "input": np.asarray(input, dtype=np.float32),
        "x_min": np.asarray(x_min, dtype=np.float32),
        "x_max": np.asarray(x_max, dtype=np.float32),
        "y_min": np.asarray(y_min, dtype=np.float32),
        "y_max": np.asarray(y_max, dtype=np.float32),
    })


# revision 38
# speedup vs baseline: 1.0343x; 1.0030x over previous
"""BoxConv2d Trainium2 kernel (8 NeuronCores, SPMD).

Math: the reference computes, per output channel k = (c, f),
    out[b,k] = interp-row(I) diff, then interp-col diff
where I is the zero-padded integral image of input[b,c].  That whole
pipeline (integral image + fractional box-edge interpolation) is linear
in the input and separable, so it collapses to two dense 128x128
matrix products per image:

    out[b,k] = A_k @ x[b,c] @ B_k^T

with banded "pixel overlap" matrices
    A_k[xo, a] = clamp(xo - a + x_max_k + 1, 0, 1)
                 - clamp(xo - a + x_min_k, 0, 1)
(the overlap length between the box row extent [xo+x_min, xo+x_max+1]
and the pixel row [a, a+1]), and likewise B_k for columns.  A/B are
built on the host from the tiny (C,F) box params; the device does pure
128-contraction matmuls on the PE array.

Sharding: the K = C*F = 128 output channels are split across 8 cores
(16 channels = 4 in_planes per core), so each core reads only its own
4 input planes and input reads are not duplicated chip-wide.

Trace-driven design notes (vs the 52us f32r baseline):
  * bf16 on the wire (x, at, bt, V, out) halves DMA bytes; fp32 PSUM
    accumulation keeps l2 rel err ~3e-3 (gate is 2e-2).
  * Only Scalar+Vector can drain PSUM on TRN2, and that drain (4.2M
    elements, ~18.6us of engine time each) is the binding resource.
    PSUM runs as two double-buffered [128,1024] 2-bank pools (pass 1 /
    pass 2), each tile filled by two 512-col matmuls and drained by
    one 1024-col cast-copy, alternated across both engines.
  * The core's DVFS grants ~20.5us of full clock per NEFF, normally
    starting ~21us in; everything before runs at half clock.  A
    dependency-free warmup burst on all four engines right after the
    preamble pulls the grant up to ~15.5-16us, which is worth 2-4us
    end to end.  (A fixed ~6us preamble and a ~8us semaphore-reset
    epilogue are framework overhead visible in the measured time.)
  * DMA engines fair-share all in-flight transfers (completion order
    is nearly issue-order-independent), so loads are few, large, and
    need-ordered on one queue: x is laid out [a,(c,b,j)] c-major so
    pass 1 of channel c waits only on its own 262KB tile.
  * Stores are 16 contiguous 256KB blocks riding the same queue,
    paced by the drain, finishing inside the full-clock window.
"""

import os
import sys

if "/opt/trn_rl_repo" not in sys.path:
    sys.path.insert(0, "/opt/trn_rl_repo")

import ml_dtypes
import numpy as np

import concourse.bass as bass  # noqa: F401
import concourse.mybir as mybir
import concourse.tile as tile
from concourse import bacc
from concourse.bass_utils import run_bass_kernel_spmd

B, C, F, H, W = 8, 32, 4, 128, 128
NCORES = 8
CPC = C // NCORES  # in_planes per core
KPC = CPC * F      # output channels per core

_DT = mybir.dt.bfloat16
_NP_DT = ml_dtypes.bfloat16

_NC_CACHE = {}
LAST_RESULT = None


def _build_nc():
    nc = bacc.Bacc(
        "TRN2", target_bir_lowering=False, debug=False, num_devices=NCORES
    )
    # x[a, (c, b, j)]: c-major so pass 1 of channel c loads only its own
    # two 131KB b-quads; rows are 1KB contiguous in DRAM
    x_p = nc.declare_dram_parameter("x", [H, CPC * B * W], _DT, isOutput=False)
    at_p = nc.declare_dram_parameter(
        "at", [H, CPC * F * H], _DT, isOutput=False)
    bt_p = nc.declare_dram_parameter(
        "bt", [W, CPC * F * W], _DT, isOutput=False)
    # outT[kl, yo, (b, xo)]: stores are contiguous 512KB kl-pair blocks
    out_p = nc.declare_dram_parameter(
        "outT", [KPC, W, B * H], _DT, isOutput=True)

    with tile.TileContext(nc) as tc:
        with (
            tc.tile_pool(name="const", bufs=1) as cpool,
            tc.tile_pool(name="vall", bufs=3) as vpool,
            tc.tile_pool(name="osb", bufs=6) as opool,
            tc.tile_pool(name="pv", bufs=2, space="PSUM") as pvpool,
            tc.tile_pool(name="po", bufs=2, space="PSUM") as popool,
        ):
            # Few large need-ordered loads: per-DMA queue cost is ~0.7us
            # regardless of size (128 descriptors either way), and the
            # DMA engines fair-share all in-flight transfers, so pairing
            # channels halves both issue time and completion spread.
            at_t2 = [cpool.tile([128, 2 * F * H], _DT, name=f"at{i}",
                                tag=f"at{i}") for i in range(2)]
            bt_t2 = [cpool.tile([128, 2 * F * W], _DT, name=f"bt{i}",
                                tag=f"bt{i}") for i in range(2)]
            # x per channel (deps are tile-granular, so per-c tiles let
            # the first pass-1 matmul wait only on at[c01] + x[c0])
            x_t = [cpool.tile([128, B * W], _DT, name=f"x{c}",
                              tag=f"x{c}") for c in range(CPC)]

            def at_c(c):
                return at_t2[c // 2][:, (c % 2) * F * H:(c % 2 + 1) * F * H]

            def bt_cf(c, f):
                o = ((c % 2) * F + f) * W
                return bt_t2[c // 2][:, o:o + W]

            def x_bc(b, c):
                return x_t[c][:, b * W:(b + 1) * W]

            at_r = at_p[:].rearrange("a (i cfx) -> a i cfx", i=2)
            bt_r = bt_p[:].rearrange("j (i cfy) -> j i cfy", i=2)
            x_r = x_p[:].rearrange("a (c bj) -> a c bj", c=CPC)

            # critical first half of the loads gets the wire alone; the
            # second half is issued from the Scalar queue, where program
            # order puts it behind the warmup copies (~2us later), so it
            # doesn't fair-share against at[c01]/x[c0] on the wire
            nc.sync.dma_start(at_t2[0][:], at_r[:, 0])
            nc.sync.dma_start(x_t[0][:], x_r[:, 0])
            nc.sync.dma_start(x_t[1][:], x_r[:, 1])
            nc.sync.dma_start(bt_t2[0][:], bt_r[:, 0])

            # dependency-free warmup matmuls: PE activity starts the
            # DVFS clock-up while the first loads are still in flight
            # dependency-free warmup: a hard burst of activity on PE,
            # GpSimd and both copy engines right after the preamble, to
            # pull the DVFS full-clock grant (fixed ~20.5us budget)
            # earlier -- pre-grant everything runs at half clock
            warm = cpool.tile([128, 512], _DT, name="warm", tag="warm")
            warm2 = cpool.tile([128, 512], _DT, name="warm2", tag="warm2")
            nc.gpsimd.memset(warm[:], 0.0)
            for w in range(3):
                nc.scalar.copy(warm2[:], warm[:])
                nc.vector.tensor_copy(warm2[:], warm[:])
                nc.gpsimd.memset(warm2[:], 0.0)
            # deferred second half of the loads (see above)
            nc.scalar.dma_start(at_t2[1][:], at_r[:, 1])
            nc.scalar.dma_start(x_t[2][:], x_r[:, 2])
            nc.scalar.dma_start(x_t[3][:], x_r[:, 3])
            nc.scalar.dma_start(bt_t2[1][:], bt_r[:, 1])
            for w in range(8):
                w_ps = pvpool.tile([128, 2 * F * H], mybir.dt.float32,
                                   name=f"wps{w}", tag="vps")
                nc.tensor.matmul(
                    w_ps[:, :512], lhsT=warm[:, :128], rhs=warm[:],
                    start=True, stop=True)

            # PSUM->SBUF copies: only Scalar and Vector can read PSUM;
            # alternate with a slight lean to the faster Scalar (17:15)
            cp_i = [0]

            def copy(dst, src):
                i = cp_i[0]
                cp_i[0] += 1
                if i % 2 == 1 and i < 30:
                    nc.vector.tensor_copy(dst, src)
                else:
                    nc.scalar.copy(dst, src)

            v_all = [None] * CPC

            def emit_pass1(c, bp):
                # two 512-col matmuls into one 2-bank PSUM tile, then one
                # 1024-col copy into V[c][j, (f, b, xo)]
                if bp == 0:
                    v_all[c] = vpool.tile(
                        [128, F * B * H], _DT, name=f"v{c}", tag="vall")
                v_ps = pvpool.tile([128, 2 * F * H], mybir.dt.float32,
                                   name=f"vps{c}{bp}", tag="vps")
                for i in range(2):
                    b = 2 * bp + i
                    nc.tensor.matmul(
                        v_ps[:, i * F * H:(i + 1) * F * H],
                        lhsT=x_bc(b, c),
                        rhs=at_c(c),
                        start=True,
                        stop=True,
                    )
                src = v_ps[:].rearrange("p (i f xo) -> p i f xo", i=2, f=F)
                dst = v_all[c][:].rearrange(
                    "p (f b xo) -> p f b xo", f=F, b=B)
                copy(dst[:, :, 2 * bp:2 * bp + 2, :],
                     src.rearrange("p i f xo -> p f i xo"))

            def emit_pass2(c, f):
                kl = c * F + f
                # O[yo, (b, xo)] for all 8 batches: 2x N=512 matmuls
                # into one 2-bank tile, one copy, one 256KB store
                o_ps = popool.tile([128, B * H], mybir.dt.float32,
                                   name=f"ops{kl}", tag="ops")
                for i in range(2):
                    nc.tensor.matmul(
                        o_ps[:, i * 512:(i + 1) * 512],
                        lhsT=bt_cf(c, f),
                        rhs=v_all[c][:, f * B * H + i * 512:
                                     f * B * H + (i + 1) * 512],
                        start=True,
                        stop=True,
                    )
                o_sb = opool.tile([128, B * H], _DT,
                                  name=f"osb{kl}", tag="osb")
                copy(o_sb[:], o_ps[:])
                nc.sync.dma_start(out_p[kl], o_sb[:])

            # software pipeline: pass 2 of channel c-1 interleaves with
            # pass 1 of channel c at matching granularity, keeping the PE
            # dense and the drain engines and store stream busy from ~10us
            for bp in range(B // 2):
                emit_pass1(0, bp)
            for c in range(1, CPC):
                for k in range(4):
                    emit_pass2(c - 1, k)
                    emit_pass1(c, k)
            for f in range(F):
                emit_pass2(CPC - 1, f)
    nc.finalize()
    return nc


def _get_nc():
    if "nc" not in _NC_CACHE:
        _NC_CACHE["nc"] = _build_nc()
    return _NC_CACHE["nc"]


def _overlap_mats(lo, hi):
    """(K, out, in) pixel-overlap matrices for a 128-wide axis."""
    t = np.arange(128, dtype=np.float64)
    d = t[:, None] - t[None, :]  # out - in
    lo = lo.astype(np.float64)[:, None, None]
    hi = hi.astype(np.float64)[:, None, None]
    m = np.clip(d[None] + hi + 1.0, 0.0, 1.0) - np.clip(d[None] + lo, 0.0, 1.0)
    return m.astype(np.float32)


def _make_in_maps(input, x_min, x_max, y_min, y_max):
    A = _overlap_mats(x_min.reshape(-1), x_max.reshape(-1))   # (K, xo, a)
    Bm = _overlap_mats(y_min.reshape(-1), y_max.reshape(-1))  # (K, yo, j)
    in_maps = []
    for m in range(NCORES):
        cs = slice(CPC * m, CPC * (m + 1))
        ks = slice(KPC * m, KPC * (m + 1))
        # x[a, (c, b, j)]
        xm = input[:, cs].transpose(2, 1, 0, 3).reshape(H, CPC * B * W)
        # at[a, (c, f, xo)] = A[k=c*F+f, xo, a]
        at = A[ks].reshape(CPC, F, H, H).transpose(3, 0, 1, 2)
        bt = Bm[ks].reshape(CPC, F, W, W).transpose(3, 0, 1, 2)
        in_maps.append({
            "x": np.ascontiguousarray(xm).astype(_NP_DT),
            "at": np.ascontiguousarray(
                at.reshape(H, CPC * F * H)).astype(_NP_DT),
            "bt": np.ascontiguousarray(
                bt.reshape(W, CPC * F * W)).astype(_NP_DT),
        })
    return in_maps


def _assemble(results):
    out = np.empty((B, C * F, H, W), np.float32)
    for m in range(NCORES):
        # outT[kl, yo, b, xo] -> out[b, kl, xo, yo]
        o = results[m]["outT"].reshape(KPC, W, B, H).astype(np.float32)
        out[:, KPC * m:KPC * (m + 1)] = o.transpose(2, 0, 3, 1)
    return out


def _run(inputs, trace=False):
    global LAST_RESULT
    nc = _get_nc()
    in_maps = _make_in_maps(**inputs)
    LAST_RESULT = run_bass_kernel_spmd(
        nc, in_maps, list(range(NCORES)), trace=trace
    )
    return _assemble(LAST_RESULT.results)


def kernel(input, x_min, x_max, y_min, y_max):
    return _run({
        "input": np.asarray(input, dtype=np.float32),
        "x_min": np.asarray(x_min, dtype=np.float32),
        "x_max": np.asarray(x_max, dtype=np.float32),
        "y_min": np.asarray(y_min, dtype=np.float32),
        "y_max": np.asarray(y_max, dtype=np.float32),
    })


# revision 40
# speedup vs baseline: 1.0396x; 1.0051x over previous
"""BoxConv2d Trainium2 kernel (8 NeuronCores, SPMD).

Math: the reference computes, per output channel k = (c, f),
    out[b,k] = interp-row(I) diff, then interp-col diff
where I is the zero-padded integral image of input[b,c].  That whole
pipeline (integral image + fractional box-edge interpolation) is linear
in the input and separable, so it collapses to two dense 128x128
matrix products per image:

    out[b,k] = A_k @ x[b,c] @ B_k^T

with banded "pixel overlap" matrices
    A_k[xo, a] = clamp(xo - a + x_max_k + 1, 0, 1)
                 - clamp(xo - a + x_min_k, 0, 1)
(the overlap length between the box row extent [xo+x_min, xo+x_max+1]
and the pixel row [a, a+1]), and likewise B_k for columns.  A/B are
built on the host from the tiny (C,F) box params; the device does pure
128-contraction matmuls on the PE array.

Sharding: the K = C*F = 128 output channels are split across 8 cores
(16 channels = 4 in_planes per core), so each core reads only its own
4 input planes and input reads are not duplicated chip-wide.

Trace-driven design notes (vs the 52us f32r baseline):
  * bf16 on the wire (x, at, bt, V, out) halves DMA bytes; fp32 PSUM
    accumulation keeps l2 rel err ~3e-3 (gate is 2e-2).
  * Only Scalar+Vector can drain PSUM on TRN2, and that drain (4.2M
    elements, ~18.6us of engine time each) is the binding resource.
    PSUM runs as two double-buffered [128,1024] 2-bank pools (pass 1 /
    pass 2), each tile filled by two 512-col matmuls and drained by
    one 1024-col cast-copy, alternated across both engines.
  * The core's DVFS grants ~20.5us of full clock per NEFF, normally
    starting ~21us in; everything before runs at half clock.  A
    dependency-free warmup burst on all four engines right after the
    preamble pulls the grant up to ~15.5-16us, which is worth 2-4us
    end to end.  (A fixed ~6us preamble and a ~8us semaphore-reset
    epilogue are framework overhead visible in the measured time.)
  * DMA engines fair-share all in-flight transfers (completion order
    is nearly issue-order-independent), so loads are few, large, and
    need-ordered on one queue: x is laid out [a,(c,b,j)] c-major so
    pass 1 of channel c waits only on its own 262KB tile.
  * Stores are 16 contiguous 256KB blocks riding the same queue,
    paced by the drain, finishing inside the full-clock window.
"""

import os
import sys

if "/opt/trn_rl_repo" not in sys.path:
    sys.path.insert(0, "/opt/trn_rl_repo")

import ml_dtypes
import numpy as np

import concourse.bass as bass  # noqa: F401
import concourse.mybir as mybir
import concourse.tile as tile
from concourse import bacc
from concourse.bass_utils import run_bass_kernel_spmd

B, C, F, H, W = 8, 32, 4, 128, 128
NCORES = 8
CPC = C // NCORES  # in_planes per core
KPC = CPC * F      # output channels per core

_DT = mybir.dt.bfloat16
_NP_DT = ml_dtypes.bfloat16

_NC_CACHE = {}
LAST_RESULT = None


def _build_nc():
    nc = bacc.Bacc(
        "TRN2", target_bir_lowering=False, debug=False, num_devices=NCORES
    )
    # x[a, (c, b, j)]: c-major so pass 1 of channel c loads only its own
    # two 131KB b-quads; rows are 1KB contiguous in DRAM
    x_p = nc.declare_dram_parameter("x", [H, CPC * B * W], _DT, isOutput=False)
    at_p = nc.declare_dram_parameter(
        "at", [H, CPC * F * H], _DT, isOutput=False)
    bt_p = nc.declare_dram_parameter(
        "bt", [W, CPC * F * W], _DT, isOutput=False)
    # outT[kl, yo, (b, xo)]: stores are contiguous 512KB kl-pair blocks
    out_p = nc.declare_dram_parameter(
        "outT", [KPC, W, B * H], _DT, isOutput=True)

    with tile.TileContext(nc) as tc:
        with (
            tc.tile_pool(name="const", bufs=1) as cpool,
            tc.tile_pool(name="vall", bufs=3) as vpool,
            tc.tile_pool(name="osb", bufs=6) as opool,
            tc.tile_pool(name="pv", bufs=2, space="PSUM") as pvpool,
            tc.tile_pool(name="po", bufs=2, space="PSUM") as popool,
        ):
            # Few large need-ordered loads: per-DMA queue cost is ~0.7us
            # regardless of size (128 descriptors either way), and the
            # DMA engines fair-share all in-flight transfers, so pairing
            # channels halves both issue time and completion spread.
            at_t2 = [cpool.tile([128, 2 * F * H], _DT, name=f"at{i}",
                                tag=f"at{i}") for i in range(2)]
            bt_t2 = [cpool.tile([128, 2 * F * W], _DT, name=f"bt{i}",
                                tag=f"bt{i}") for i in range(2)]
            # x per channel (deps are tile-granular, so per-c tiles let
            # the first pass-1 matmul wait only on at[c01] + x[c0])
            x_t = [cpool.tile([128, B * W], _DT, name=f"x{c}",
                              tag=f"x{c}") for c in range(CPC)]

            def at_c(c):
                return at_t2[c // 2][:, (c % 2) * F * H:(c % 2 + 1) * F * H]

            def bt_cf(c, f):
                o = ((c % 2) * F + f) * W
                return bt_t2[c // 2][:, o:o + W]

            def x_bc(b, c):
                return x_t[c][:, b * W:(b + 1) * W]

            at_r = at_p[:].rearrange("a (i cfx) -> a i cfx", i=2)
            bt_r = bt_p[:].rearrange("j (i cfy) -> j i cfy", i=2)
            x_r = x_p[:].rearrange("a (c bj) -> a c bj", c=CPC)

            nc.sync.dma_start(at_t2[0][:], at_r[:, 0])
            nc.sync.dma_start(x_t[0][:], x_r[:, 0])
            nc.sync.dma_start(x_t[1][:], x_r[:, 1])
            nc.sync.dma_start(bt_t2[0][:], bt_r[:, 0])
            nc.sync.dma_start(at_t2[1][:], at_r[:, 1])
            nc.sync.dma_start(x_t[2][:], x_r[:, 2])
            nc.sync.dma_start(x_t[3][:], x_r[:, 3])
            nc.sync.dma_start(bt_t2[1][:], bt_r[:, 1])

            # dependency-free warmup matmuls: PE activity starts the
            # DVFS clock-up while the first loads are still in flight
            # dependency-free warmup: a hard burst of activity on PE,
            # GpSimd and both copy engines right after the preamble, to
            # pull the DVFS full-clock grant (fixed ~20.5us budget)
            # earlier -- pre-grant everything runs at half clock
            warm = cpool.tile([128, 512], _DT, name="warm", tag="warm")
            warm2 = cpool.tile([128, 512], _DT, name="warm2", tag="warm2")
            nc.gpsimd.memset(warm[:], 0.0)
            for w in range(3):
                nc.scalar.copy(warm2[:], warm[:])
                nc.vector.tensor_copy(warm2[:], warm[:])
                nc.gpsimd.memset(warm2[:], 0.0)
            for w in range(6):
                w_ps = pvpool.tile([128, 2 * F * H], mybir.dt.float32,
                                   name=f"wps{w}", tag="vps")
                nc.tensor.matmul(
                    w_ps[:, :512], lhsT=warm[:, :128], rhs=warm[:],
                    start=True, stop=True)

            # PSUM->SBUF copies: only Scalar and Vector can read PSUM;
            # alternate with a slight lean to the faster Scalar (17:15)
            cp_i = [0]

            def copy(dst, src):
                i = cp_i[0]
                cp_i[0] += 1
                if i % 2 == 1 and i < 30:
                    nc.vector.tensor_copy(dst, src)
                else:
                    nc.scalar.copy(dst, src)

            v_all = [None] * CPC

            def emit_pass1(c, bp):
                # two 512-col matmuls into one 2-bank PSUM tile, then one
                # 1024-col copy into V[c][j, (f, b, xo)]
                if bp == 0:
                    v_all[c] = vpool.tile(
                        [128, F * B * H], _DT, name=f"v{c}", tag="vall")
                v_ps = pvpool.tile([128, 2 * F * H], mybir.dt.float32,
                                   name=f"vps{c}{bp}", tag="vps")
                for i in range(2):
                    b = 2 * bp + i
                    nc.tensor.matmul(
                        v_ps[:, i * F * H:(i + 1) * F * H],
                        lhsT=x_bc(b, c),
                        rhs=at_c(c),
                        start=True,
                        stop=True,
                    )
                src = v_ps[:].rearrange("p (i f xo) -> p i f xo", i=2, f=F)
                dst = v_all[c][:].rearrange(
                    "p (f b xo) -> p f b xo", f=F, b=B)
                copy(dst[:, :, 2 * bp:2 * bp + 2, :],
                     src.rearrange("p i f xo -> p f i xo"))

            def emit_pass2(c, f):
                kl = c * F + f
                # O[yo, (b, xo)] for all 8 batches: 2x N=512 matmuls
                # into one 2-bank tile, one copy, one 256KB store
                o_ps = popool.tile([128, B * H], mybir.dt.float32,
                                   name=f"ops{kl}", tag="ops")
                for i in range(2):
                    nc.tensor.matmul(
                        o_ps[:, i * 512:(i + 1) * 512],
                        lhsT=bt_cf(c, f),
                        rhs=v_all[c][:, f * B * H + i * 512:
                                     f * B * H + (i + 1) * 512],
                        start=True,
                        stop=True,
                    )
                o_sb = opool.tile([128, B * H], _DT,
                                  name=f"osb{kl}", tag="osb")
                copy(o_sb[:], o_ps[:])
                nc.sync.dma_start(out_p[kl], o_sb[:])

            # software pipeline: pass 2 of channel c-1 interleaves with
            # pass 1 of channel c at matching granularity, keeping the PE
            # dense and the drain engines and store stream busy from ~10us
            for bp in range(B // 2):
                emit_pass1(0, bp)
            for c in range(1, CPC):
                for k in range(4):
                    emit_pass2(c - 1, k)
                    emit_pass1(c, k)
            for f in range(F):
                emit_pass2(CPC - 1, f)
    nc.finalize()
    return nc


def _get_nc():
    if "nc" not in _NC_CACHE:
        _NC_CACHE["nc"] = _build_nc()
    return _NC_CACHE["nc"]


def _overlap_mats(lo, hi):
    """(K, out, in) pixel-overlap matrices for a 128-wide axis."""
    t = np.arange(128, dtype=np.float64)
    d = t[:, None] - t[None, :]  # out - in
    lo = lo.astype(np.float64)[:, None, None]
    hi = hi.astype(np.float64)[:, None, None]
    m = np.clip(d[None] + hi + 1.0, 0.0, 1.0) - np.clip(d[None] + lo, 0.0, 1.0)
    return m.astype(np.float32)


def _make_in_maps(input, x_min, x_max, y_min, y_max):
    A = _overlap_mats(x_min.reshape(-1), x_max.reshape(-1))   # (K, xo, a)
    Bm = _overlap_mats(y_min.reshape(-1), y_max.reshape(-1))  # (K, yo, j)
    in_maps = []
    for m in range(NCORES):
        cs = slice(CPC * m, CPC * (m + 1))
        ks = slice(KPC * m, KPC * (m + 1))
        # x[a, (c, b, j)]
        xm = input[:, cs].transpose(2, 1, 0, 3).reshape(H, CPC * B * W)
        # at[a, (c, f, xo)] = A[k=c*F+f, xo, a]
        at = A[ks].reshape(CPC, F, H, H).transpose(3, 0, 1, 2)
        bt = Bm[ks].reshape(CPC, F, W, W).transpose(3, 0, 1, 2)
        in_maps.append({
            "x": np.ascontiguousarray(xm).astype(_NP_DT),
            "at": np.ascontiguousarray(
                at.reshape(H, CPC * F * H)).astype(_NP_DT),
            "bt": np.ascontiguousarray(
                bt.reshape(W, CPC * F * W)).astype(_NP_DT),
        })
    return in_maps


def _assemble(results):
    out = np.empty((B, C * F, H, W), np.float32)
    for m in range(NCORES):
        # outT[kl, yo, b, xo] -> out[b, kl, xo, yo]
        o = results[m]["outT"].reshape(KPC, W, B, H).astype(np.float32)
        out[:, KPC * m:KPC * (m + 1)] = o.transpose(2, 0, 3, 1)
    return out


def _run(inputs, trace=False):
    global LAST_RESULT
    nc = _get_nc()
    in_maps = _make_in_maps(**inputs)
    LAST_RESULT = run_bass_kernel_spmd(
        nc, in_maps, list(range(NCORES)), trace=trace
    )
    return _assemble(LAST_RESULT.results)


def kernel(input, x_min, x_max, y_min, y_max):
    return _run({
        "input": np.asarray(input, dtype=np.float32),
        "x_min": np.asarray(x_min, dtype=np.float32),
        "x_max": np.asarray(x_max, dtype=np.float32),
        "y_min": np.asarray(y_min, dtype=np.float32),
        "y_max": np.asarray(y_max, dtype=np.float32),
    })


# revision 42
# speedup vs baseline: 1.0579x; 1.0176x over previous
"""BoxConv2d Trainium2 kernel (8 NeuronCores, SPMD).

Math: the reference computes, per output channel k = (c, f),
    out[b,k] = interp-row(I) diff, then interp-col diff
where I is the zero-padded integral image of input[b,c].  That whole
pipeline (integral image + fractional box-edge interpolation) is linear
in the input and separable, so it collapses to two dense 128x128
matrix products per image:

    out[b,k] = A_k @ x[b,c] @ B_k^T

with banded "pixel overlap" matrices
    A_k[xo, a] = clamp(xo - a + x_max_k + 1, 0, 1)
                 - clamp(xo - a + x_min_k, 0, 1)
(the overlap length between the box row extent [xo+x_min, xo+x_max+1]
and the pixel row [a, a+1]), and likewise B_k for columns.  A/B are
built on the host from the tiny (C,F) box params; the device does pure
128-contraction matmuls on the PE array.

Sharding: the K = C*F = 128 output channels are split across 8 cores
(16 channels = 4 in_planes per core), so each core reads only its own
4 input planes and input reads are not duplicated chip-wide.

Trace-driven design notes (vs the 52us f32r baseline):
  * bf16 on the wire (x, at, bt, V, out) halves DMA bytes; fp32 PSUM
    accumulation keeps l2 rel err ~3e-3 (gate is 2e-2).
  * Only Scalar+Vector can drain PSUM on TRN2, and that drain (4.2M
    elements, ~18.6us of engine time each) is the binding resource.
    PSUM runs as two double-buffered [128,1024] 2-bank pools (pass 1 /
    pass 2), each tile filled by two 512-col matmuls and drained by
    one 1024-col cast-copy, alternated across both engines.
  * The core's DVFS grants ~20.5us of full clock per NEFF, normally
    starting ~21us in; everything before runs at half clock.  A
    dependency-free warmup burst on all four engines right after the
    preamble pulls the grant up to ~15.5-16us, which is worth 2-4us
    end to end.  (A fixed ~6us preamble and a ~8us semaphore-reset
    epilogue are framework overhead visible in the measured time.)
  * DMA engines fair-share all in-flight transfers (completion order
    is nearly issue-order-independent), so loads are few, large, and
    need-ordered on one queue: x is laid out [a,(c,b,j)] c-major so
    pass 1 of channel c waits only on its own 262KB tile.
  * Stores are 16 contiguous 256KB blocks riding the same queue,
    paced by the drain, finishing inside the full-clock window.
"""

import os
import sys

if "/opt/trn_rl_repo" not in sys.path:
    sys.path.insert(0, "/opt/trn_rl_repo")

import ml_dtypes
import numpy as np

import concourse.bass as bass  # noqa: F401
import concourse.mybir as mybir
import concourse.tile as tile
from concourse import bacc
from concourse.bass_utils import run_bass_kernel_spmd

B, C, F, H, W = 8, 32, 4, 128, 128
NCORES = 8
CPC = C // NCORES  # in_planes per core
KPC = CPC * F      # output channels per core

_DT = mybir.dt.bfloat16
_NP_DT = ml_dtypes.bfloat16

_NC_CACHE = {}
LAST_RESULT = None


def _build_nc():
    nc = bacc.Bacc(
        "TRN2", target_bir_lowering=False, debug=False, num_devices=NCORES
    )
    # x[a, (c, b, j)]: c-major so pass 1 of channel c loads only its own
    # two 131KB b-quads; rows are 1KB contiguous in DRAM
    x_p = nc.declare_dram_parameter("x", [H, CPC * B * W], _DT, isOutput=False)
    at_p = nc.declare_dram_parameter(
        "at", [H, CPC * F * H], _DT, isOutput=False)
    bt_p = nc.declare_dram_parameter(
        "bt", [W, CPC * F * W], _DT, isOutput=False)
    # outT[kl, yo, (b, xo)]: stores are contiguous 512KB kl-pair blocks
    out_p = nc.declare_dram_parameter(
        "outT", [KPC, W, B * H], _DT, isOutput=True)

    with tile.TileContext(nc) as tc:
        with (
            tc.tile_pool(name="const", bufs=1) as cpool,
            tc.tile_pool(name="vall", bufs=5) as vpool,
            tc.tile_pool(name="osb", bufs=6) as opool,
            tc.tile_pool(name="pv", bufs=2, space="PSUM") as pvpool,
            tc.tile_pool(name="po", bufs=2, space="PSUM") as popool,
        ):
            # Few large need-ordered loads: per-DMA queue cost is ~0.7us
            # regardless of size (128 descriptors either way), and the
            # DMA engines fair-share all in-flight transfers, so pairing
            # channels halves both issue time and completion spread.
            at_t2 = [cpool.tile([128, 2 * F * H], _DT, name=f"at{i}",
                                tag=f"at{i}") for i in range(2)]
            bt_t2 = [cpool.tile([128, 2 * F * W], _DT, name=f"bt{i}",
                                tag=f"bt{i}") for i in range(2)]
            # x per channel (deps are tile-granular, so per-c tiles let
            # the first pass-1 matmul wait only on at[c01] + x[c0])
            x_t = [cpool.tile([128, B * W], _DT, name=f"x{c}",
                              tag=f"x{c}") for c in range(CPC)]

            def at_c(c):
                return at_t2[c // 2][:, (c % 2) * F * H:(c % 2 + 1) * F * H]

            def bt_cf(c, f):
                o = ((c % 2) * F + f) * W
                return bt_t2[c // 2][:, o:o + W]

            def x_bc(b, c):
                return x_t[c][:, b * W:(b + 1) * W]

            at_r = at_p[:].rearrange("a (i cfx) -> a i cfx", i=2)
            bt_r = bt_p[:].rearrange("j (i cfy) -> j i cfy", i=2)
            x_r = x_p[:].rearrange("a (c bj) -> a c bj", c=CPC)

            nc.sync.dma_start(at_t2[0][:], at_r[:, 0])
            nc.sync.dma_start(x_t[0][:], x_r[:, 0])
            nc.sync.dma_start(x_t[1][:], x_r[:, 1])
            nc.sync.dma_start(bt_t2[0][:], bt_r[:, 0])
            nc.sync.dma_start(at_t2[1][:], at_r[:, 1])
            nc.sync.dma_start(x_t[2][:], x_r[:, 2])
            nc.sync.dma_start(x_t[3][:], x_r[:, 3])
            nc.sync.dma_start(bt_t2[1][:], bt_r[:, 1])

            # dependency-free warmup matmuls: PE activity starts the
            # DVFS clock-up while the first loads are still in flight
            # dependency-free warmup: a hard burst of activity on PE,
            # GpSimd and both copy engines right after the preamble, to
            # pull the DVFS full-clock grant (fixed ~20.5us budget)
            # earlier -- pre-grant everything runs at half clock
            warm = cpool.tile([128, 512], _DT, name="warm", tag="warm")
            warm2 = cpool.tile([128, 512], _DT, name="warm2", tag="warm2")
            nc.gpsimd.memset(warm[:], 0.0)
            for w in range(3):
                nc.scalar.copy(warm2[:], warm[:])
                nc.vector.tensor_copy(warm2[:], warm[:])
                nc.gpsimd.memset(warm2[:], 0.0)
            for w in range(6):
                w_ps = pvpool.tile([128, 2 * F * H], mybir.dt.float32,
                                   name=f"wps{w}", tag="vps")
                nc.tensor.matmul(
                    w_ps[:, :512], lhsT=warm[:, :128], rhs=warm[:],
                    start=True, stop=True)

            # PSUM->SBUF copies: only Scalar and Vector can read PSUM;
            # alternate with a slight lean to the faster Scalar (17:15)
            cp_i = [0]

            def copy(dst, src):
                i = cp_i[0]
                cp_i[0] += 1
                if i % 2 == 1 and i < 30:
                    nc.vector.tensor_copy(dst, src)
                else:
                    nc.scalar.copy(dst, src)

            # V lives in per-half-batch tiles so each pass-2 matmul only
            # waits on that half's two V-copies (deps are tile-granular),
            # letting pass 2 start earlier and the drain feed sooner
            v_h = [[None] * 2 for _ in range(CPC)]

            def emit_pass1(c, bp):
                # two 512-col matmuls into one 2-bank PSUM tile, then one
                # 1024-col copy into V[c][h][j, (f, bh, xo)]
                h = bp // 2
                if bp % 2 == 0:
                    v_h[c][h] = vpool.tile(
                        [128, F * 4 * H], _DT, name=f"v{c}{h}", tag="vall")
                v_ps = pvpool.tile([128, 2 * F * H], mybir.dt.float32,
                                   name=f"vps{c}{bp}", tag="vps")
                for i in range(2):
                    b = 2 * bp + i
                    nc.tensor.matmul(
                        v_ps[:, i * F * H:(i + 1) * F * H],
                        lhsT=x_bc(b, c),
                        rhs=at_c(c),
                        start=True,
                        stop=True,
                    )
                src = v_ps[:].rearrange("p (i f xo) -> p i f xo", i=2, f=F)
                dst = v_h[c][h][:].rearrange(
                    "p (f b xo) -> p f b xo", f=F, b=4)
                q = (bp % 2) * 2
                copy(dst[:, :, q:q + 2, :],
                     src.rearrange("p i f xo -> p f i xo"))

            def emit_pass2(c, f):
                kl = c * F + f
                # O[yo, (b, xo)] for all 8 batches: 2x N=512 matmuls
                # into one 2-bank tile, one copy, one 256KB store
                o_ps = popool.tile([128, B * H], mybir.dt.float32,
                                   name=f"ops{kl}", tag="ops")
                for i in range(2):
                    nc.tensor.matmul(
                        o_ps[:, i * 512:(i + 1) * 512],
                        lhsT=bt_cf(c, f),
                        rhs=v_h[c][i][:, f * 4 * H:(f + 1) * 4 * H],
                        start=True,
                        stop=True,
                    )
                o_sb = opool.tile([128, B * H], _DT,
                                  name=f"osb{kl}", tag="osb")
                copy(o_sb[:], o_ps[:])
                nc.sync.dma_start(out_p[kl], o_sb[:])

            # software pipeline: pass 2 of channel c-1 interleaves with
            # pass 1 of channel c at matching granularity, keeping the PE
            # dense and the drain engines and store stream busy from ~10us
            for bp in range(B // 2):
                emit_pass1(0, bp)
            for c in range(1, CPC):
                for k in range(4):
                    emit_pass2(c - 1, k)
                    emit_pass1(c, k)
            for f in range(F):
                emit_pass2(CPC - 1, f)
    nc.finalize()
    return nc


def _get_nc():
    if "nc" not in _NC_CACHE:
        _NC_CACHE["nc"] = _build_nc()
    return _NC_CACHE["nc"]


def _overlap_mats(lo, hi):
    """(K, out, in) pixel-overlap matrices for a 128-wide axis."""
    t = np.arange(128, dtype=np.float64)
    d = t[:, None] - t[None, :]  # out - in
    lo = lo.astype(np.float64)[:, None, None]
    hi = hi.astype(np.float64)[:, None, None]
    m = np.clip(d[None] + hi + 1.0, 0.0, 1.0) - np.clip(d[None] + lo, 0.0, 1.0)
    return m.astype(np.float32)


def _make_in_maps(input, x_min, x_max, y_min, y_max):
    A = _overlap_mats(x_min.reshape(-1), x_max.reshape(-1))   # (K, xo, a)
    Bm = _overlap_mats(y_min.reshape(-1), y_max.reshape(-1))  # (K, yo, j)
    in_maps = []
    for m in range(NCORES):
        cs = slice(CPC * m, CPC * (m + 1))
        ks = slice(KPC * m, KPC * (m + 1))
        # x[a, (c, b, j)]
        xm = input[:, cs].transpose(2, 1, 0, 3).reshape(H, CPC * B * W)
        # at[a, (c, f, xo)] = A[k=c*F+f, xo, a]
        at = A[ks].reshape(CPC, F, H, H).transpose(3, 0, 1, 2)
        bt = Bm[ks].reshape(CPC, F, W, W).transpose(3, 0, 1, 2)
        in_maps.append({
            "x": np.ascontiguousarray(xm).astype(_NP_DT),
            "at": np.ascontiguousarray(
                at.reshape(H, CPC * F * H)).astype(_NP_DT),
            "bt": np.ascontiguousarray(
                bt.reshape(W, CPC * F * W)).astype(_NP_DT),
        })
    return in_maps


def _assemble(results):
    out = np.empty((B, C * F, H, W), np.float32)
    for m in range(NCORES):
        # outT[kl, yo, b, xo] -> out[b, kl, xo, yo]
        o = results[m]["outT"].reshape(KPC, W, B, H).astype(np.float32)
        out[:, KPC * m:KPC * (m + 1)] = o.transpose(2, 0, 3, 1)
    return out


def _run(inputs, trace=False):
    global LAST_RESULT
    nc = _get_nc()
    in_maps = _make_in_maps(**inputs)
    LAST_RESULT = run_bass_kernel_spmd(
        nc, in_maps, list(range(NCORES)), trace=trace
    )
    return _assemble(LAST_RESULT.results)


def kernel(input, x_min, x_max, y_min, y_max):
    return _run({
        "input": np.asarray(input, dtype=np.float32),
        "x_min": np.asarray(x_min, dtype=np.float32),
        "x_max": np.asarray(x_max, dtype=np.float32),
        "y_min": np.asarray(y_min, dtype=np.float32),
        "y_max": np.asarray(y_max, dtype=np.float32),
    })
